# revision 20
# baseline (speedup 1.0000x reference)
"""
Trainium2 Bass kernel for batched cross-attention:
  context[b] = softmax(q[b] @ tokens[b].T / sqrt(d)) @ tokens[b]
with x_latent (tokens) [16, 4096, 768] f32, prompts_latent (q) [16, 64, 768] f32.

Sharding: data-parallel over the batch dim — 16 batches / 8 cores = 2 per core.

Per-core algorithm (bf16 matmuls, f32 accumulation):
  - host pre-transposes: qT [768, 64] and T^T [768, 4096] (both bf16), and
    ships tokens in natural layout T [4096, 768] bf16 as well.
  - mm1: S[64, 512-group] = qT.T @ T^T-slice, accumulated over 6 d-chunks of 128.
  - softmax without max-subtraction (scores ~ N(0,1) after scaling; exp is safe):
    P = exp(S * scale) on ACT with fused per-row accumulation of the sums.
  - P chunks [64, 128] are PE-transposed to P^T [128, 64] (needed because the
    second matmul contracts over n, which must be the partition dim).
  - mm2: O[64, 768] += P^T-tile.T @ T-tile, accumulated over 32 n-tiles.
  - O rows divided by the softmax sums at the end (DVE), stored as f32.
"""

import os
import sys

import numpy as np

for _p in ("/opt/trn_rl_repo", "/root/.axon_site/_ro/trn_rl_repo"):
    if os.path.isdir(_p) and _p not in sys.path:
        sys.path.append(_p)

import ml_dtypes
from contextlib import ExitStack

import concourse.bass as bass
import concourse.mybir as mybir
import concourse.tile as tile
from concourse import bacc
from concourse.bass_utils import run_bass_kernel_spmd
from concourse.masks import make_identity

BF16 = ml_dtypes.bfloat16

N_CORES = 8
B_TOTAL = 16
BPC = B_TOTAL // N_CORES  # batches per core
N = 4096  # tokens
D = 768   # latent dim
P = 64    # prompts
DC = D // 128   # d-chunks of 128 (contraction tiles for mm1)
NT = N // 128   # n-tiles of 128
G = N // 512    # groups of 512 columns for mm1/softmax
SCALE = float(D) ** -0.5

_cached_nc = None


def build_bass_program() -> bass.Bass:
    nc = bacc.Bacc("TRN2", target_bir_lowering=False, debug=False)
    qt = nc.declare_dram_parameter("qt", [BPC, D, P], mybir.dt.bfloat16, isOutput=False)
    tt = nc.declare_dram_parameter("tt", [BPC, D, N], mybir.dt.bfloat16, isOutput=False)
    tn = nc.declare_dram_parameter("tn", [BPC, N, D], mybir.dt.bfloat16, isOutput=False)
    out = nc.declare_dram_parameter("out", [BPC, P, D], mybir.dt.float32, isOutput=True)

    with tile.TileContext(nc) as tc, ExitStack() as ctx:
        singles = ctx.enter_context(tc.tile_pool(name="singles", bufs=1))
        qt_pool = ctx.enter_context(tc.tile_pool(name="qt", bufs=3))
        tt_pool = ctx.enter_context(tc.tile_pool(name="ttg", bufs=6))
        tn_pool = ctx.enter_context(tc.tile_pool(name="tnt", bufs=6))
        p_pool = ctx.enter_context(tc.tile_pool(name="pexp", bufs=3))
        pt_pool = ctx.enter_context(tc.tile_pool(name="ptT", bufs=8))
        sums_pool = ctx.enter_context(tc.tile_pool(name="sums", bufs=2))
        o_pool = ctx.enter_context(tc.tile_pool(name="osb", bufs=2))

        psum_s = ctx.enter_context(tc.tile_pool(name="psum_s", bufs=3, space="PSUM"))
        psum_pt = ctx.enter_context(tc.tile_pool(name="psum_pt", bufs=3, space="PSUM"))
        psum_o = ctx.enter_context(tc.tile_pool(name="psum_o", bufs=1, space="PSUM"))

        ident = singles.tile([P, P], mybir.dt.bfloat16)
        make_identity(nc, ident)

        # HAM warmup: ~4us of dependency-free matmuls at kernel start so the
        # PE clock gate reaches K=8/8 while the first DMAs are in flight.
        warm_ps = psum_s.tile([P, 512], mybir.dt.float32, tag="s_ps")
        for _ in range(30):
            nc.tensor.matmul(
                warm_ps[:, 0:P], lhsT=ident, rhs=ident, start=True, stop=True
            )

        for b in range(BPC):
            # qT chunks: [128, c, 64], partition = d within chunk
            qt_t = qt_pool.tile([128, DC, P], mybir.dt.bfloat16)
            nc.sync.dma_start(
                out=qt_t, in_=qt[b].rearrange("(c p) m -> p c m", p=128)
            )

            sums = sums_pool.tile([P, G], mybir.dt.float32)
            o_a = psum_o.tile([P, 512], mybir.dt.float32)
            o_b = psum_o.tile([P, 256], mybir.dt.float32)

            tt_r = tt[b].rearrange("(c p) n -> p c n", p=128)
            tn_r = tn[b].rearrange("(g t p) d -> p g t d", t=4, p=128)

            def transpose_stage(p_sb, g):
                # PE transposes of the 4 P chunks + DVE copies to SBUF.
                pts = []
                for j in range(4):
                    pt_ps = psum_pt.tile([128, P], mybir.dt.bfloat16)
                    nc.tensor.transpose(
                        pt_ps, p_sb[:, j * 128:(j + 1) * 128], ident
                    )
                    pts.append(pt_ps)
                out = []
                for j in range(4):
                    pt_sb = pt_pool.tile([128, P], mybir.dt.bfloat16)
                    nc.vector.tensor_copy(pt_sb, pts[j])
                    out.append(pt_sb)
                return out

            def mm2_stage(pt_sbs, tn_g, g):
                for j in range(4):
                    nt = g * 4 + j
                    nc.tensor.matmul(
                        o_a,
                        lhsT=pt_sbs[j],
                        rhs=tn_g[:, j, 0:512],
                        start=(nt == 0),
                        stop=(nt == NT - 1),
                    )
                    nc.tensor.matmul(
                        o_b,
                        lhsT=pt_sbs[j],
                        rhs=tn_g[:, j, 512:768],
                        start=(nt == 0),
                        stop=(nt == NT - 1),
                    )

            # Two-stage software pipeline: PE program order per iteration is
            # [mm1(g)] [transposes(g-1)] [mm2(g-2)], so the DVE copies of
            # group g-1 complete well before mm2(g-1) issues and PE never
            # stalls on ACT/DVE.
            tr_pend = None   # (p_sb, g) awaiting transpose stage
            mm2_pend = None  # (pt_sbs, tn_g, g) awaiting mm2 stage
            for g in range(G):
                tt_g = tt_pool.tile([128, DC, 512], mybir.dt.bfloat16)
                nc.sync.dma_start(out=tt_g, in_=tt_r[:, :, g * 512:(g + 1) * 512])

                tn_g = tn_pool.tile([128, 4, D], mybir.dt.bfloat16)
                nc.sync.dma_start(out=tn_g, in_=tn_r[:, g])

                s_ps = psum_s.tile([P, 512], mybir.dt.float32)
                for c in range(DC):
                    nc.tensor.matmul(
                        s_ps,
                        lhsT=qt_t[:, c, :],
                        rhs=tt_g[:, c, :],
                        start=(c == 0),
                        stop=(c == DC - 1),
                    )

                # P = exp(S * scale), cast to bf16. Chunked so the PE
                # transposes can start after the first 128 columns instead of
                # waiting out the full-width activation. Row sums on DVE.
                p_sb = p_pool.tile([P, 512], mybir.dt.bfloat16)
                for j in range(4):
                    nc.scalar.activation(
                        out=p_sb[:, j * 128:(j + 1) * 128],
                        in_=s_ps[:, j * 128:(j + 1) * 128],
                        func=mybir.ActivationFunctionType.Exp,
                        scale=SCALE,
                    )
                nc.vector.reduce_sum(
                    sums[:, g:g + 1], p_sb, axis=mybir.AxisListType.X
                )

                if tr_pend is not None:
                    pt_sbs = transpose_stage(*tr_pend)
                    if mm2_pend is not None:
                        mm2_stage(*mm2_pend)
                    mm2_pend = (pt_sbs, tn_g_prev, tr_pend[1])
                tr_pend = (p_sb, g)
                tn_g_prev = tn_g
            pt_sbs = transpose_stage(*tr_pend)
            if mm2_pend is not None:
                mm2_stage(*mm2_pend)
            mm2_stage(pt_sbs, tn_g_prev, tr_pend[1])

            tot = sums_pool.tile([P, 1], mybir.dt.float32)
            nc.vector.reduce_sum(tot, sums, axis=mybir.AxisListType.X)
            rec = sums_pool.tile([P, 1], mybir.dt.float32)
            nc.vector.reciprocal(rec, tot)

            o_sb = o_pool.tile([P, D], mybir.dt.float32)
            nc.vector.tensor_scalar_mul(o_sb[:, 0:512], o_a, rec)
            nc.vector.tensor_scalar_mul(o_sb[:, 512:768], o_b, rec)
            nc.sync.dma_start(out=out[b], in_=o_sb)

    nc.compile()
    return nc


def _get_nc() -> bass.Bass:
    global _cached_nc
    if _cached_nc is None:
        _cached_nc = build_bass_program()
    return _cached_nc


def _make_in_maps(x_latent: np.ndarray, prompts_latent: np.ndarray):
    tn_h = np.ascontiguousarray(x_latent.astype(BF16))            # [16, N, D]
    tt_h = np.ascontiguousarray(tn_h.transpose(0, 2, 1))          # [16, D, N]
    qt_h = np.ascontiguousarray(prompts_latent.astype(BF16).transpose(0, 2, 1))
    return [
        {
            "qt": qt_h[c * BPC:(c + 1) * BPC],
            "tt": tt_h[c * BPC:(c + 1) * BPC],
            "tn": tn_h[c * BPC:(c + 1) * BPC],
        }
        for c in range(N_CORES)
    ]


def run(x_latent: np.ndarray, prompts_latent: np.ndarray, trace: bool = False):
    """Run on all 8 cores; returns (output [16, 64, 768] f32, BassKernelResults)."""
    nc = _get_nc()
    in_maps = _make_in_maps(np.asarray(x_latent), np.asarray(prompts_latent))
    res = run_bass_kernel_spmd(nc, in_maps, list(range(N_CORES)), trace=trace)
    out = np.concatenate([np.asarray(r["out"]) for r in res.results], axis=0)
    return out.astype(np.float32), res


def kernel(x_latent: np.ndarray, prompts_latent: np.ndarray) -> np.ndarray:
    out, _ = run(x_latent, prompts_latent, trace=False)
    return out
